# revision 29
# baseline (speedup 1.0000x reference)
"""Causal multi-head self-attention (RoPE) Trainium2 Bass kernel.

Problem: x:(4,2048,1024), Wq/Wk/Wv:(1024,1024), Wo:(1024,1024), bo:(1024,)
  q,k,v = split_heads(x@W*), rope(q), rope(k), causal softmax(q k^T/8) v, @Wo+bo

Sharding: head-parallel across 8 cores. Core c owns heads {2c, 2c+1} for all
4 batches: it computes q/k/v projections against the 128-column weight slice,
attention for its heads, and a partial output projection against the matching
128-row slice of Wo. Host sums the 8 partial (8192,1024) fp16 outputs and
adds bo.

On-core layout:
  Q^T/K^T (128 x 2048/batch): rows = [h0 d-evens(32), h0 d-odds(32), h1 ...]
    (NeoX-style d-permutation, folded into the host-permuted weight columns;
     valid because q and k get the same permutation and qk^T is d-invariant)
  RoPE: Q <- P*cos - (P2@(P*sin2)), computed with DVE reads straight from the
    projection PSUM; P2 swaps even/odd halves per head (PE matmul), sin2
    carries the sign.
  V is projected token-major directly: per 128-token tile, lhsT = xT chunk,
    rhs = Wv chunk, accumulated over the 8 contraction chunks; PSUM result is
    cast once into Vb[tok, h, tile, 0:64], with a ones column at col 64
    (memset) for the softmax denominator.
  S^T tiles (tj x ti) = K^T.T @ Q^T per head (K=64 row-group pairs run
    concurrently on the PE).
  A = exp(0.125*S^T) (ACT, both heads in one instruction); straddle tiles get
    a post-exp fp16 0/1-triangle multiply (no PSUM read-modify-write).
  O~^T (65 x ti) accumulated = [V|1].T @ A over tj chunks; row 64 = softmax
    denominators. Normalize: PSUM->fp16 avs copy, SBUF-SBUF DMA repartition
    of the denominator row, lane-parallel reciprocal, repartition back to one
    row, GpSimd partition_broadcast, DVE multiply -> O^T (128 x 2048) fp16.
  y partial (128t x 1024) fp16 = O^T-chunk.T @ Wo-slice, evacuated half on
    DVE / half on ACT, DMA'd to DRAM as fp16.
"""

import numpy as np

B, T, C = 4, 2048, 1024
H, D = 16, 64
N_CORES = 8
BT = B * T
SCALE = 0.125  # D**-0.5

TRACE = False            # set True (e.g. from test.py) to capture an NTFF trace
LAST_RESULT = None       # BassKernelResults of the most recent run

_BUILT = None            # cached bass program


# --------------------------------------------------------------------------
# workaround: this walrus build rejects >1 semaphore wait per instruction
def _split_sem_waits(nc, max_waits=1):
    import concourse.mybir as mybir

    n = 0
    for f in nc.m.functions:
        for bb in f.blocks:
            insts = bb.instructions
            idx = 0
            while idx < len(insts):
                i = insts[idx]
                si = getattr(i, "sync_info", None)
                if si is not None and si.on_wait and len(si.on_wait) > max_waits:
                    waits = list(si.on_wait)
                    extra, keep = waits[:-max_waits], waits[-max_waits:]
                    si.on_wait = keep
                    pos = idx
                    for j in range(0, len(extra), max_waits):
                        n += 1
                        nd = mybir.InstNoOp(name=f"I-waitsplit-{n}", ins=[], outs=[])
                        nd.engine = i.engine
                        nd.sync_info = mybir.SyncInfo(
                            on_wait=extra[j : j + max_waits], on_update=[]
                        )
                        insts.insert(pos, nd)
                        pos += 1
                    idx = pos
                idx += 1


def _install_ntff_hook():
    """The image's antenv lacks axon_hooks; synthesize it so trace=True works."""
    import sys
    import types

    if "antenv.axon_hooks" in sys.modules:
        return
    import antenv

    state = {"hook": None}
    mod = types.ModuleType("antenv.axon_hooks")
    mod.get_axon_ntff_profile_hook = lambda: state["hook"]
    mod.set_axon_ntff_profile_hook = lambda h: state.__setitem__("hook", h)
    sys.modules["antenv.axon_hooks"] = mod
    antenv.axon_hooks = mod
    try:
        from trn_agent_boot.trn_boot import _ntff_profile_via_ctypes

        state["hook"] = _ntff_profile_via_ctypes("/opt/axon/libaxon_pjrt.so")
    except Exception:
        state["hook"] = None


# --------------------------------------------------------------------------
def _build():
    import concourse.bass as bass
    import concourse.mybir as mybir
    from concourse.tile import TileContext

    F = mybir.dt.float32
    MD = mybir.dt.float16  # matmul operand dtype
    MULT = mybir.AluOpType.mult
    SUB = mybir.AluOpType.subtract
    EXP = mybir.ActivationFunctionType.Exp

    nc = bass.Bass()

    xT = nc.dram_tensor("xT", (C, BT), MD, kind="ExternalInput")
    wq = nc.dram_tensor("wq", (C, 128), MD, kind="ExternalInput")
    wk = nc.dram_tensor("wk", (C, 128), MD, kind="ExternalInput")
    wv = nc.dram_tensor("wv", (C, 128), MD, kind="ExternalInput")
    wo = nc.dram_tensor("wo", (128, C), MD, kind="ExternalInput")
    cosd = nc.dram_tensor("cos", (128, T), MD, kind="ExternalInput")
    sind = nc.dram_tensor("sin2", (128, T), MD, kind="ExternalInput")
    p2d = nc.dram_tensor("p2", (128, 128), MD, kind="ExternalInput")
    trid = nc.dram_tensor("tri2x", (128, 256), MD, kind="ExternalInput")
    y = nc.dram_tensor("y", (BT, C), MD, kind="ExternalOutput")

    with TileContext(nc) as tc:
        with (
            tc.tile_pool(name="const", bufs=1) as cst,
            tc.tile_pool(name="xt", bufs=3) as xtp,
            tc.tile_pool(name="qt", bufs=2) as qp,
            tc.tile_pool(name="kt", bufs=2) as kp,
            tc.tile_pool(name="vt", bufs=2) as vp,
            tc.tile_pool(name="ot", bufs=2) as op_,
            tc.tile_pool(name="tmp", bufs=4) as tmp,
            tc.tile_pool(name="at", bufs=6) as ap_,
            tc.tile_pool(name="bc", bufs=4) as bcp,
            tc.tile_pool(name="avs", bufs=6) as avsp,
            tc.tile_pool(name="rr", bufs=12) as rp,
            tc.tile_pool(name="ys", bufs=4) as ysp,
            tc.tile_pool(name="sps", bufs=2, space="PSUM") as sps,
            tc.tile_pool(name="stp", bufs=2, space="PSUM") as stp,
            tc.tile_pool(name="avp", bufs=1, space="PSUM") as avp,
        ):
            # ---- constants (batched DMAs; ordered so the first projection
            # matmuls are not queued behind yet-unneeded loads) --------------
            wq_t = cst.tile([128, 8, 128], MD)
            wk_t = cst.tile([128, 8, 128], MD)
            wv_t = cst.tile([128, 8, 128], MD)
            nc.sync.dma_start(
                out=wq_t, in_=wq[:, :].rearrange("(k p) c -> p k c", p=128))
            nc.sync.dma_start(
                out=wk_t, in_=wk[:, :].rearrange("(k p) c -> p k c", p=128))
            nc.sync.dma_start(
                out=wv_t, in_=wv[:, :].rearrange("(k p) c -> p k c", p=128))
            # prefetch the first token block so the first projection matmuls
            # are not queued behind the remaining constant loads
            xt00 = xtp.tile([128, 8, 512], MD, name="xt")
            nc.sync.dma_start(
                out=xt00, in_=xT[:, 0:512].rearrange("(k p) c -> p k c", p=128))
            cos_t = cst.tile([128, T], MD)
            nc.sync.dma_start(out=cos_t, in_=cosd[:, :])
            sin_t = cst.tile([128, T], MD)
            nc.sync.dma_start(out=sin_t, in_=sind[:, :])
            p2_t = cst.tile([128, 128], MD)
            nc.sync.dma_start(out=p2_t, in_=p2d[:, :])
            tri_t = cst.tile([128, 256], MD)  # [tri01 | tri01] for head pairs
            nc.sync.dma_start(out=tri_t, in_=trid[:, :])
            ones64 = cst.tile([1, 64], MD)  # broadcast helper (lhsT)
            nc.vector.memset(ones64[0:1, :], 1.0)
            wo_t = cst.tile([128, C], MD)  # first needed ~i-block 1
            nc.sync.dma_start(out=wo_t, in_=wo[:, :])

            QKV = {}  # b -> (Qb, Kb, Vb)
            XT_PRE = {(0, 0): xt00}  # (b, nb) -> prefetched xt tile

            def xt_load(b, nb):
                g0 = b * T + nb * 512
                xt = xtp.tile([128, 8, 512], MD, name="xt")
                nc.sync.dma_start(
                    out=xt,
                    in_=xT[:, g0 : g0 + 512].rearrange("(k p) c -> p k c", p=128),
                )
                return xt

            def phase_a_alloc(b):
                Qb = qp.tile([128, T], MD, name="Qb")
                Kb = kp.tile([128, T], MD, name="Kb")
                # Vb token-major: per head 65 cols = [d 0..63 | ones]
                Vb = vp.tile([128, 2, 16, 65], MD, name="Vb")
                QKV[b] = (Qb, Kb, Vb)
                nc.vector.memset(Vb[:, :, :, 64], 1.0)

            def phase_a_unit(b, nb):
                Qb, Kb, Vb = QKV[b]
                cols = slice(nb * 512, (nb + 1) * 512)
                xt = XT_PRE.pop((b, nb), None)
                if xt is None:
                    xt = xt_load(b, nb)
                for W, dst in ((wq_t, Qb), (wk_t, Kb)):
                    ps = sps.tile([128, 512], F, tag="s", name="ps")
                    for k in range(8):
                        nc.tensor.matmul(
                            ps[:, :], lhsT=W[:, k, :], rhs=xt[:, k, :],
                            start=(k == 0), stop=(k == 7),
                        )
                    # rope: dst = ps*cos - P2@(ps*sin2)
                    #   (P2@ (q.sin2))[p] = -q~[p]*sin2[p], since sin2 is
                    #    antisymmetric and cos symmetric under the pair swap
                    qs = tmp.tile([128, 512], MD, name="qs")
                    nc.vector.tensor_tensor(qs[:, :], ps[:, :],
                                            sin_t[:, cols], MULT)
                    nc.vector.tensor_tensor(dst[:, cols], ps[:, :],
                                            cos_t[:, cols], MULT)
                    rot = sps.tile([128, 512], F, tag="s", name="rot")
                    nc.tensor.matmul(rot[:, :], lhsT=p2_t[:, :], rhs=qs[:, :],
                                     start=True, stop=True)
                    nc.vector.tensor_tensor(dst[:, cols], dst[:, cols],
                                            rot[:, :], SUB)
                # V token-major: per 128-token tile accumulate the 8 k-chunks
                for tl in range(4):
                    tt = nb * 4 + tl
                    vps = sps.tile([128, 128], F, tag="s", name="vps")
                    for k in range(8):
                        nc.tensor.matmul(
                            vps[:, :], lhsT=xt[:, k, tl * 128 : (tl + 1) * 128],
                            rhs=wv_t[:, k, :],
                            start=(k == 0), stop=(k == 7),
                        )
                    nc.vector.tensor_copy(
                        Vb[:, :, tt, 0:64],
                        vps[:, :].rearrange("p (h f) -> p h f", h=2),
                    )

            def y_unit(b, Ob, i):
                # output projection for the 4 token-tiles of ti-block i;
                # one batched 1 MB DMA for the whole 512-token block
                ysb = ysp.tile([128, 4, 1024], MD, name="ysb")
                for tl in range(4):
                    tt = 4 * i + tl
                    lhs = Ob[:, tt * 128 : (tt + 1) * 128]
                    for nh in (0, 1):
                        yps = sps.tile([128, 512], F, tag="s", name="yps")
                        nc.tensor.matmul(
                            yps[:, :], lhsT=lhs,
                            rhs=wo_t[:, nh * 512 : (nh + 1) * 512],
                            start=True, stop=True,
                        )
                        if nh == 0:
                            nc.vector.tensor_copy(
                                ysb[:, tl, 0:512], yps[:, :])
                        else:
                            nc.scalar.copy(ysb[:, tl, 512:1024], yps[:, :])
                r0 = b * T + i * 512
                nc.sync.dma_start(
                    out=y[r0 : r0 + 512, :].rearrange("(t p) c -> p t c", p=128),
                    in_=ysb[:, :, :],
                )

            def norm_evac(av):
                # evacuate both heads' accumulators (incl denom row 64) to
                # fp32 SBUF in one pass, releasing the PSUM banks promptly
                avs = avsp.tile([65, 2, 512], F, name="avs")
                nc.vector.tensor_copy(avs[:, :, :], av[0:65, :, :])
                return avs

            def norm_pre(avs):
                # denominators (both heads): SBUF-SBUF repartition ->
                # lane-parallel reciprocal -> fp16 -> repartition back to one
                # row. No PE instructions: emitted immediately so the DMA
                # latency runs while the deferred PE stage waits its turn.
                srt = rp.tile([128, 8], F, name="srt")
                nc.sync.dma_start(out=srt[:, :], in_=avs[64:65, :, :])
                rt = rp.tile([128, 8], F, name="rt")
                nc.vector.reciprocal(rt[:, :], srt[:, :])
                rth = rp.tile([128, 8], MD, name="rth")
                nc.vector.tensor_copy(rth[:, :], rt[:, :])
                rdr = rp.tile([1, 1024], MD, name="rdr")
                nc.sync.dma_start(out=rdr[0:1, :], in_=rth[:, :])
                return rdr

            def norm_fin(rdr, avs, Ob, i, h):
                # PE ones-matmul broadcast into 64 PSUM partitions, then
                # normalize into Ob
                bct = sps.tile([128, 512], F, tag="s", name="bct")
                nc.tensor.matmul(
                    bct[0:64, :], lhsT=ones64[0:1, :],
                    rhs=rdr[0:1, h * 512 : (h + 1) * 512],
                    start=True, stop=True,
                )
                nc.vector.tensor_tensor(
                    Ob[64 * h : 64 * h + 64, i * 512 : (i + 1) * 512],
                    avs[0:64, h, :], bct[0:64, :], MULT,
                )

            def phase_d(b, filler=None, pre=None, carry=()):
                Qb, Kb, Vb = QKV[b]
                Ob = op_.tile([128, T], MD, name="Ob")
                defer = list(carry)  # deferred PE-stage closures (norm_fin /
                # y_unit), emitted one per j-slot a ti-block later so the PE
                # queue never heads on a normalize DMA
                for i in range(4):
                    if pre is not None:
                        pre(i)
                    av = avp.tile([128, 2, 512], F, tag="av", name="av")
                    nch = 4 * i + 4
                    sts = {}

                    def emit_st(j):
                        delta = j * 128 - i * 512
                        nl = 512 - max(0, delta)
                        off = 512 - nl
                        st = stp.tile([128, 2, 512], F, name="st")
                        for h in (0, 1):
                            hs = slice(64 * h, 64 * h + 64)
                            nc.tensor.matmul(
                                st[:, h, 0:nl],
                                lhsT=Kb[hs, j * 128 : (j + 1) * 128],
                                rhs=Qb[hs, i * 512 + off : (i + 1) * 512],
                                start=True, stop=True,
                            )
                        sts[j] = (st, off, nl, delta >= 0)

                    LAG = 1
                    for j in range(min(LAG, nch)):
                        emit_st(j)
                    for j in range(nch):
                        if j + LAG < nch:
                            emit_st(j + LAG)
                        if defer and j in (1, 2, 3):
                            defer.pop(0)()
                        st, off, nl, straddle = sts.pop(j)
                        A = ap_.tile([128, 2, 512], MD, name="A")
                        nc.scalar.activation(
                            A[:, :, 0:nl], st[:, :, 0:nl], EXP, scale=SCALE)
                        if straddle:  # mask the diagonal triangle (post-exp)
                            # on GpSimd: it is otherwise idle and this keeps
                            # the hot DVE out of the A critical path
                            nc.gpsimd.tensor_tensor(
                                A[:, :, 0:128], A[:, :, 0:128],
                                tri_t[:, :].rearrange("p (a c) -> p a c", a=2),
                                MULT)
                        for h in (0, 1):
                            nc.tensor.matmul(
                                av[0:65, h, off:512],
                                lhsT=Vb[:, h, j, :],
                                rhs=A[:, h, 0:nl],
                                start=(j == 0), stop=(j == nch - 1),
                                skip_group_check=True,
                            )
                    avs = norm_evac(av)
                    rdr = norm_pre(avs)
                    if i < 3 or b == B - 1:
                        # inline: the chain hides under subsequent blocks
                        for h in (0, 1):
                            norm_fin(rdr, avs, Ob, i, h)
                        defer.append(lambda i2=i: y_unit(b, Ob, i2))
                    else:
                        # last i-block of a non-final batch: defer the PE
                        # stage + y_unit into the next batch's stream
                        defer.extend([
                            lambda r=rdr, a=avs, h2=h:
                                norm_fin(r, a, Ob, 3, h2)
                            for h in (0, 1)
                        ])
                        defer.append(lambda: y_unit(b, Ob, 3))
                    if filler is not None:
                        filler(i)
                if b == B - 1:
                    for fn in defer:
                        fn()
                    defer = []
                return defer

            phase_a_alloc(0)
            carry = ()
            for b in range(B):
                if b + 1 < B:
                    phase_a_alloc(b + 1)
                    fil = (lambda i, nb=b + 1: phase_a_unit(nb, i))
                else:
                    fil = None
                # batch 0's projection blocks are emitted just-in-time ahead
                # of the attention block that first needs them
                pre = (lambda i: phase_a_unit(0, i)) if b == 0 else None
                carry = phase_d(b, filler=fil, pre=pre, carry=carry)

    _split_sem_waits(nc)
    return nc


# --------------------------------------------------------------------------
def _host_inputs(x, Wq, Wk, Wv):
    """Per-core input dicts (all shared arrays built once)."""
    BF = np.float16
    xT = np.ascontiguousarray(
        np.asarray(x, dtype=np.float32).reshape(BT, C).T).astype(BF)

    # NeoX d-permutation within each head: evens then odds
    dperm = np.concatenate([np.arange(0, D, 2), np.arange(1, D, 2)])

    inv_freq = (1.0 / (10000.0 ** (np.arange(0, D, 2) / D))).astype(np.float64)
    pos = np.arange(T, dtype=np.float64)
    ang = pos[None, :] * inv_freq[:, None]  # (32, T)
    cos32 = np.cos(ang).astype(np.float32)
    sin32 = np.sin(ang).astype(np.float32)
    cos_t = np.tile(np.vstack([cos32, cos32]), (2, 1))  # (128, T)
    sin_t = np.tile(np.vstack([-sin32, sin32]), (2, 1))  # (128, T), sign folded

    p2 = np.zeros((128, 128), dtype=np.float32)
    for hb in (0, 64):
        for i2 in range(32):
            p2[hb + i2, hb + 32 + i2] = 1.0
            p2[hb + 32 + i2, hb + i2] = 1.0

    # 0/1 mask: 0 where k-token (partition) > q-token (col), else 1
    tri = np.where(
        np.arange(128)[None, :] < np.arange(128)[:, None], 0.0, 1.0
    ).astype(np.float32)
    tri2x = np.concatenate([tri, tri], axis=1)  # (128, 256)

    Wq = np.asarray(Wq, dtype=np.float32)
    Wk = np.asarray(Wk, dtype=np.float32)
    Wv = np.asarray(Wv, dtype=np.float32)

    in_maps = []
    for c in range(N_CORES):
        sl = slice(128 * c, 128 * (c + 1))
        wq_c = Wq[:, sl].reshape(C, 2, D)[:, :, dperm].reshape(C, 128)
        wk_c = Wk[:, sl].reshape(C, 2, D)[:, :, dperm].reshape(C, 128)
        in_maps.append({
            "xT": xT,
            "wq": np.ascontiguousarray(wq_c).astype(BF),
            "wk": np.ascontiguousarray(wk_c).astype(BF),
            "wv": np.ascontiguousarray(Wv[:, sl]).astype(BF),
            "wo": None,  # set below
            "cos": cos_t.astype(BF),
            "sin2": sin_t.astype(BF),
            "p2": p2.astype(BF),
            "tri2x": tri2x.astype(BF),
        })
    return in_maps


def kernel(x, Wq, Wk, Wv, Wo, bo):
    global _BUILT, LAST_RESULT
    from concourse.bass_utils import run_bass_kernel_spmd

    if TRACE:
        _install_ntff_hook()

    if _BUILT is None:
        _BUILT = _build()
    nc = _BUILT

    in_maps = _host_inputs(x, Wq, Wk, Wv)
    Wo = np.asarray(Wo, dtype=np.float32)
    for c in range(N_CORES):
        in_maps[c]["wo"] = np.ascontiguousarray(
            Wo[128 * c : 128 * (c + 1), :]).astype(np.float16)

    last_err = None
    for attempt in range(3):
        try:
            res = run_bass_kernel_spmd(
                nc, in_maps, core_ids=list(range(N_CORES)), trace=TRACE
            )
            break
        except Exception as e:  # transient NRT device errors: retry
            last_err = e
            import time as _time

            _time.sleep(2.0)
    else:
        raise last_err
    LAST_RESULT = res

    acc = res.results[0]["y"].astype(np.float32)
    for c in range(1, N_CORES):
        acc = acc + res.results[c]["y"].astype(np.float32)
    out = acc + np.asarray(bo, dtype=np.float32)[None, :]
    return out.reshape(B, T, C)


# revision 31
# speedup vs baseline: 1.1060x; 1.1060x over previous
"""Causal multi-head self-attention (RoPE) Trainium2 Bass kernel.

Problem: x:(4,2048,1024), Wq/Wk/Wv:(1024,1024), Wo:(1024,1024), bo:(1024,)
  q,k,v = split_heads(x@W*), rope(q), rope(k), causal softmax(q k^T/8) v, @Wo+bo

Sharding: head-parallel across 8 cores. Core c owns heads {2c, 2c+1} for all
4 batches: it computes q/k/v projections against the 128-column weight slice,
attention for its heads, and a partial output projection against the matching
128-row slice of Wo. Host sums the 8 partial (8192,1024) fp16 outputs and
adds bo.

On-core layout:
  Q^T/K^T (128 x 2048/batch): rows = [h0 d-evens(32), h0 d-odds(32), h1 ...]
    (NeoX-style d-permutation, folded into the host-permuted weight columns;
     valid because q and k get the same permutation and qk^T is d-invariant)
  RoPE: Q <- P*cos - (P2@(P*sin2)), computed with DVE reads straight from the
    projection PSUM; P2 swaps even/odd halves per head (PE matmul), sin2
    carries the sign.
  V is projected token-major directly: per 128-token tile, lhsT = xT chunk,
    rhs = Wv chunk, accumulated over the 8 contraction chunks; PSUM result is
    cast once into Vb[tok, h, tile, 0:64], with a ones column at col 64
    (memset) for the softmax denominator.
  S^T tiles (tj x ti) = K^T.T @ Q^T per head (K=64 row-group pairs run
    concurrently on the PE).
  A = exp(0.125*S^T) (ACT, both heads in one instruction); straddle tiles get
    a post-exp fp16 0/1-triangle multiply (no PSUM read-modify-write).
  O~^T (65 x ti) accumulated = [V|1].T @ A over tj chunks; row 64 = softmax
    denominators. Normalize: PSUM->fp16 avs copy, SBUF-SBUF DMA repartition
    of the denominator row, lane-parallel reciprocal, repartition back to one
    row, GpSimd partition_broadcast, DVE multiply -> O^T (128 x 2048) fp16.
  y partial (128t x 1024) fp16 = O^T-chunk.T @ Wo-slice, evacuated half on
    DVE / half on ACT, DMA'd to DRAM as fp16.
"""

import numpy as np

B, T, C = 4, 2048, 1024
H, D = 16, 64
N_CORES = 8
BT = B * T
SCALE = 0.125  # D**-0.5

TRACE = False            # set True (e.g. from test.py) to capture an NTFF trace
LAST_RESULT = None       # BassKernelResults of the most recent run

_BUILT = None            # cached bass program


# --------------------------------------------------------------------------
# workaround: this walrus build rejects >1 semaphore wait per instruction
def _split_sem_waits(nc, max_waits=1):
    import concourse.mybir as mybir

    n = 0
    for f in nc.m.functions:
        for bb in f.blocks:
            insts = bb.instructions
            idx = 0
            while idx < len(insts):
                i = insts[idx]
                si = getattr(i, "sync_info", None)
                if si is not None and si.on_wait and len(si.on_wait) > max_waits:
                    waits = list(si.on_wait)
                    extra, keep = waits[:-max_waits], waits[-max_waits:]
                    si.on_wait = keep
                    pos = idx
                    for j in range(0, len(extra), max_waits):
                        n += 1
                        nd = mybir.InstNoOp(name=f"I-waitsplit-{n}", ins=[], outs=[])
                        nd.engine = i.engine
                        nd.sync_info = mybir.SyncInfo(
                            on_wait=extra[j : j + max_waits], on_update=[]
                        )
                        insts.insert(pos, nd)
                        pos += 1
                    idx = pos
                idx += 1


def _install_ntff_hook():
    """The image's antenv lacks axon_hooks; synthesize it so trace=True works."""
    import sys
    import types

    if "antenv.axon_hooks" in sys.modules:
        return
    import antenv

    state = {"hook": None}
    mod = types.ModuleType("antenv.axon_hooks")
    mod.get_axon_ntff_profile_hook = lambda: state["hook"]
    mod.set_axon_ntff_profile_hook = lambda h: state.__setitem__("hook", h)
    sys.modules["antenv.axon_hooks"] = mod
    antenv.axon_hooks = mod
    try:
        from trn_agent_boot.trn_boot import _ntff_profile_via_ctypes

        state["hook"] = _ntff_profile_via_ctypes("/opt/axon/libaxon_pjrt.so")
    except Exception:
        state["hook"] = None


# --------------------------------------------------------------------------
def _build():
    import concourse.bass as bass
    import concourse.mybir as mybir
    from concourse.tile import TileContext

    F = mybir.dt.float32
    MD = mybir.dt.float16  # matmul operand dtype
    MULT = mybir.AluOpType.mult
    SUB = mybir.AluOpType.subtract
    EXP = mybir.ActivationFunctionType.Exp

    nc = bass.Bass()

    xT = nc.dram_tensor("xT", (C, BT), MD, kind="ExternalInput")
    wq = nc.dram_tensor("wq", (C, 128), MD, kind="ExternalInput")
    wk = nc.dram_tensor("wk", (C, 128), MD, kind="ExternalInput")
    wv = nc.dram_tensor("wv", (C, 128), MD, kind="ExternalInput")
    wo = nc.dram_tensor("wo", (128, C), MD, kind="ExternalInput")
    cosd = nc.dram_tensor("cos", (128, T), MD, kind="ExternalInput")
    sind = nc.dram_tensor("sin2", (128, T), MD, kind="ExternalInput")
    p2d = nc.dram_tensor("p2", (128, 128), MD, kind="ExternalInput")
    trid = nc.dram_tensor("tri2x", (128, 256), MD, kind="ExternalInput")
    y = nc.dram_tensor("y", (BT, C), MD, kind="ExternalOutput")

    with TileContext(nc) as tc:
        with (
            tc.tile_pool(name="const", bufs=1) as cst,
            tc.tile_pool(name="xt", bufs=3) as xtp,
            tc.tile_pool(name="qt", bufs=2) as qp,
            tc.tile_pool(name="kt", bufs=2) as kp,
            tc.tile_pool(name="vt", bufs=2) as vp,
            tc.tile_pool(name="ot", bufs=2) as op_,
            tc.tile_pool(name="tmp", bufs=4) as tmp,
            tc.tile_pool(name="at", bufs=6) as ap_,
            tc.tile_pool(name="bc", bufs=4) as bcp,
            tc.tile_pool(name="avs", bufs=6) as avsp,
            tc.tile_pool(name="rr", bufs=12) as rp,
            tc.tile_pool(name="ys", bufs=4) as ysp,
            tc.tile_pool(name="sps", bufs=2, space="PSUM") as sps,
            tc.tile_pool(name="stp", bufs=2, space="PSUM") as stp,
            tc.tile_pool(name="avp", bufs=1, space="PSUM") as avp,
        ):
            # ---- constants (batched DMAs; ordered so the first projection
            # matmuls are not queued behind yet-unneeded loads) --------------
            wq_t = cst.tile([128, 8, 128], MD)
            wk_t = cst.tile([128, 8, 128], MD)
            wv_t = cst.tile([128, 8, 128], MD)
            nc.sync.dma_start(
                out=wq_t, in_=wq[:, :].rearrange("(k p) c -> p k c", p=128))
            nc.sync.dma_start(
                out=wk_t, in_=wk[:, :].rearrange("(k p) c -> p k c", p=128))
            nc.sync.dma_start(
                out=wv_t, in_=wv[:, :].rearrange("(k p) c -> p k c", p=128))
            # prefetch the first token block so the first projection matmuls
            # are not queued behind the remaining constant loads
            xt00 = xtp.tile([128, 8, 512], MD, name="xt")
            nc.sync.dma_start(
                out=xt00, in_=xT[:, 0:512].rearrange("(k p) c -> p k c", p=128))
            cos_t = cst.tile([128, T], MD)
            nc.sync.dma_start(out=cos_t, in_=cosd[:, :])
            sin_t = cst.tile([128, T], MD)
            nc.sync.dma_start(out=sin_t, in_=sind[:, :])
            p2_t = cst.tile([128, 128], MD)
            nc.sync.dma_start(out=p2_t, in_=p2d[:, :])
            tri_t = cst.tile([128, 256], MD)  # [tri01 | tri01] for head pairs
            nc.sync.dma_start(out=tri_t, in_=trid[:, :])
            ones64 = cst.tile([1, 64], MD)  # broadcast helper (lhsT)
            nc.vector.memset(ones64[0:1, :], 1.0)
            wo_t = cst.tile([128, C], MD)  # first needed ~i-block 1
            nc.sync.dma_start(out=wo_t, in_=wo[:, :])

            QKV = {}  # b -> (Qb, Kb, Vb)
            XT_PRE = {(0, 0): xt00}  # (b, nb) -> prefetched xt tile

            def xt_load(b, nb):
                g0 = b * T + nb * 512
                xt = xtp.tile([128, 8, 512], MD, name="xt")
                nc.sync.dma_start(
                    out=xt,
                    in_=xT[:, g0 : g0 + 512].rearrange("(k p) c -> p k c", p=128),
                )
                return xt

            def phase_a_alloc(b):
                Qb = qp.tile([128, T], MD, name="Qb")
                Kb = kp.tile([128, T], MD, name="Kb")
                # Vb token-major: per head 65 cols = [d 0..63 | ones]
                Vb = vp.tile([128, 2, 16, 65], MD, name="Vb")
                QKV[b] = (Qb, Kb, Vb)
                nc.vector.memset(Vb[:, :, :, 64], 1.0)

            def phase_a_unit(b, nb):
                Qb, Kb, Vb = QKV[b]
                cols = slice(nb * 512, (nb + 1) * 512)
                xt = XT_PRE.pop((b, nb), None)
                if xt is None:
                    xt = xt_load(b, nb)
                for W, dst in ((wq_t, Qb), (wk_t, Kb)):
                    ps = sps.tile([128, 512], F, tag="s", name="ps")
                    for k in range(8):
                        nc.tensor.matmul(
                            ps[:, :], lhsT=W[:, k, :], rhs=xt[:, k, :],
                            start=(k == 0), stop=(k == 7),
                        )
                    # rope: dst = ps*cos - P2@(ps*sin2)
                    #   (P2@ (q.sin2))[p] = -q~[p]*sin2[p], since sin2 is
                    #    antisymmetric and cos symmetric under the pair swap
                    qs = tmp.tile([128, 512], MD, name="qs")
                    nc.vector.tensor_tensor(qs[:, :], ps[:, :],
                                            sin_t[:, cols], MULT)
                    nc.vector.tensor_tensor(dst[:, cols], ps[:, :],
                                            cos_t[:, cols], MULT)
                    rot = sps.tile([128, 512], F, tag="s", name="rot")
                    nc.tensor.matmul(rot[:, :], lhsT=p2_t[:, :], rhs=qs[:, :],
                                     start=True, stop=True)
                    nc.vector.tensor_tensor(dst[:, cols], dst[:, cols],
                                            rot[:, :], SUB)
                # V token-major: per 128-token tile accumulate the 8 k-chunks
                for tl in range(4):
                    tt = nb * 4 + tl
                    vps = sps.tile([128, 128], F, tag="s", name="vps")
                    for k in range(8):
                        nc.tensor.matmul(
                            vps[:, :], lhsT=xt[:, k, tl * 128 : (tl + 1) * 128],
                            rhs=wv_t[:, k, :],
                            start=(k == 0), stop=(k == 7),
                        )
                    nc.vector.tensor_copy(
                        Vb[:, :, tt, 0:64],
                        vps[:, :].rearrange("p (h f) -> p h f", h=2),
                    )

            def y_unit(b, Ob, i):
                # output projection for the 4 token-tiles of ti-block i;
                # one batched 1 MB DMA for the whole 512-token block
                ysb = ysp.tile([128, 4, 1024], MD, name="ysb")
                for tl in range(4):
                    tt = 4 * i + tl
                    lhs = Ob[:, tt * 128 : (tt + 1) * 128]
                    for nh in (0, 1):
                        yps = sps.tile([128, 512], F, tag="s", name="yps")
                        nc.tensor.matmul(
                            yps[:, :], lhsT=lhs,
                            rhs=wo_t[:, nh * 512 : (nh + 1) * 512],
                            start=True, stop=True,
                        )
                        if nh == 0:
                            nc.vector.tensor_copy(
                                ysb[:, tl, 0:512], yps[:, :])
                        else:
                            nc.scalar.copy(ysb[:, tl, 512:1024], yps[:, :])
                r0 = b * T + i * 512
                nc.sync.dma_start(
                    out=y[r0 : r0 + 512, :].rearrange("(t p) c -> p t c", p=128),
                    in_=ysb[:, :, :],
                )

            def norm_evac(av):
                # evacuate both heads' accumulators (incl denom row 64) to
                # fp32 SBUF in one pass, releasing the PSUM banks promptly
                avs = avsp.tile([65, 2, 512], F, name="avs")
                nc.vector.tensor_copy(avs[:, :, :], av[0:65, :, :])
                return avs

            def norm_pre(avs):
                # denominators (both heads): SBUF-SBUF repartition ->
                # lane-parallel reciprocal -> fp16 -> repartition back to one
                # row. No PE instructions: emitted immediately so the DMA
                # latency runs while the deferred PE stage waits its turn.
                srt = rp.tile([128, 8], F, name="srt")
                nc.sync.dma_start(out=srt[:, :], in_=avs[64:65, :, :])
                rt = rp.tile([128, 8], F, name="rt")
                nc.vector.reciprocal(rt[:, :], srt[:, :])
                rth = rp.tile([128, 8], MD, name="rth")
                nc.vector.tensor_copy(rth[:, :], rt[:, :])
                rdr = rp.tile([1, 1024], MD, name="rdr")
                nc.sync.dma_start(out=rdr[0:1, :], in_=rth[:, :])
                return rdr

            def norm_fin(rdr, avs, Ob, i, h):
                # PE ones-matmul broadcast into 64 PSUM partitions, then
                # normalize into Ob
                bct = sps.tile([128, 512], F, tag="s", name="bct")
                nc.tensor.matmul(
                    bct[0:64, :], lhsT=ones64[0:1, :],
                    rhs=rdr[0:1, h * 512 : (h + 1) * 512],
                    start=True, stop=True,
                )
                nc.vector.tensor_tensor(
                    Ob[64 * h : 64 * h + 64, i * 512 : (i + 1) * 512],
                    avs[0:64, h, :], bct[0:64, :], MULT,
                )

            def phase_d(b, filler=None, pre=None, carry=()):
                Qb, Kb, Vb = QKV[b]
                Ob = op_.tile([128, T], MD, name="Ob")
                defer = list(carry)  # deferred PE-stage closures (norm_fin /
                # y_unit), emitted one per j-slot a ti-block later so the PE
                # queue never heads on a normalize DMA
                for i in range(4):
                    if pre is not None:
                        pre(i)
                    av = avp.tile([128, 2, 512], F, tag="av", name="av")
                    nch = 4 * i + 4
                    sts = {}

                    def emit_st(j):
                        delta = j * 128 - i * 512
                        nl = 512 - max(0, delta)
                        off = 512 - nl
                        st = stp.tile([128, 2, 512], F, name="st")
                        for h in (0, 1):
                            hs = slice(64 * h, 64 * h + 64)
                            nc.tensor.matmul(
                                st[:, h, 0:nl],
                                lhsT=Kb[hs, j * 128 : (j + 1) * 128],
                                rhs=Qb[hs, i * 512 + off : (i + 1) * 512],
                                start=True, stop=True,
                            )
                        sts[j] = (st, off, nl, delta >= 0)

                    LAG = 1
                    for j in range(min(LAG, nch)):
                        emit_st(j)
                    for j in range(nch):
                        if j + LAG < nch:
                            emit_st(j + LAG)
                        if defer and j in (1, 2, 3):
                            defer.pop(0)()
                        st, off, nl, straddle = sts.pop(j)
                        A = ap_.tile([128, 2, 512], MD, name="A")
                        nc.scalar.activation(
                            A[:, :, 0:nl], st[:, :, 0:nl], EXP, scale=SCALE)
                        if straddle:  # mask the diagonal triangle (post-exp)
                            # on GpSimd: it is otherwise idle and this keeps
                            # the hot DVE out of the A critical path
                            nc.gpsimd.tensor_tensor(
                                A[:, :, 0:128], A[:, :, 0:128],
                                tri_t[:, :].rearrange("p (a c) -> p a c", a=2),
                                MULT)
                        for h in (0, 1):
                            nc.tensor.matmul(
                                av[0:65, h, off:512],
                                lhsT=Vb[:, h, j, :],
                                rhs=A[:, h, 0:nl],
                                start=(j == 0), stop=(j == nch - 1),
                                skip_group_check=True,
                            )
                    avs = norm_evac(av)
                    rdr = norm_pre(avs)
                    if b > 0 and i < 3:
                        # steady state: inline — the chain hides under
                        # subsequent blocks and keeps PE duty moderate
                        for h in (0, 1):
                            norm_fin(rdr, avs, Ob, i, h)
                        defer.append(lambda i2=i: y_unit(b, Ob, i2))
                    else:
                        # batch 0 (PE runs caught-up with emission, any
                        # serial chain is a PE stall) and each batch's last
                        # i-block: defer the PE stage + y_unit into the
                        # subsequent emission stream
                        defer.extend([
                            lambda r=rdr, a=avs, i2=i, h2=h:
                                norm_fin(r, a, Ob, i2, h2)
                            for h in (0, 1)
                        ])
                        defer.append(lambda i2=i: y_unit(b, Ob, i2))
                    if filler is not None:
                        filler(i)
                if b == B - 1:
                    for fn in defer:
                        fn()
                    defer = []
                return defer

            phase_a_alloc(0)
            carry = ()
            for b in range(B):
                if b + 1 < B:
                    phase_a_alloc(b + 1)
                    fil = (lambda i, nb=b + 1: phase_a_unit(nb, i))
                else:
                    fil = None
                # batch 0's projection blocks are emitted just-in-time ahead
                # of the attention block that first needs them
                pre = (lambda i: phase_a_unit(0, i)) if b == 0 else None
                carry = phase_d(b, filler=fil, pre=pre, carry=carry)

    _split_sem_waits(nc)
    return nc


# --------------------------------------------------------------------------
def _host_inputs(x, Wq, Wk, Wv):
    """Per-core input dicts (all shared arrays built once)."""
    BF = np.float16
    xT = np.ascontiguousarray(
        np.asarray(x, dtype=np.float32).reshape(BT, C).T).astype(BF)

    # NeoX d-permutation within each head: evens then odds
    dperm = np.concatenate([np.arange(0, D, 2), np.arange(1, D, 2)])

    inv_freq = (1.0 / (10000.0 ** (np.arange(0, D, 2) / D))).astype(np.float64)
    pos = np.arange(T, dtype=np.float64)
    ang = pos[None, :] * inv_freq[:, None]  # (32, T)
    cos32 = np.cos(ang).astype(np.float32)
    sin32 = np.sin(ang).astype(np.float32)
    cos_t = np.tile(np.vstack([cos32, cos32]), (2, 1))  # (128, T)
    sin_t = np.tile(np.vstack([-sin32, sin32]), (2, 1))  # (128, T), sign folded

    p2 = np.zeros((128, 128), dtype=np.float32)
    for hb in (0, 64):
        for i2 in range(32):
            p2[hb + i2, hb + 32 + i2] = 1.0
            p2[hb + 32 + i2, hb + i2] = 1.0

    # 0/1 mask: 0 where k-token (partition) > q-token (col), else 1
    tri = np.where(
        np.arange(128)[None, :] < np.arange(128)[:, None], 0.0, 1.0
    ).astype(np.float32)
    tri2x = np.concatenate([tri, tri], axis=1)  # (128, 256)

    Wq = np.asarray(Wq, dtype=np.float32)
    Wk = np.asarray(Wk, dtype=np.float32)
    Wv = np.asarray(Wv, dtype=np.float32)

    in_maps = []
    for c in range(N_CORES):
        sl = slice(128 * c, 128 * (c + 1))
        wq_c = Wq[:, sl].reshape(C, 2, D)[:, :, dperm].reshape(C, 128)
        wk_c = Wk[:, sl].reshape(C, 2, D)[:, :, dperm].reshape(C, 128)
        in_maps.append({
            "xT": xT,
            "wq": np.ascontiguousarray(wq_c).astype(BF),
            "wk": np.ascontiguousarray(wk_c).astype(BF),
            "wv": np.ascontiguousarray(Wv[:, sl]).astype(BF),
            "wo": None,  # set below
            "cos": cos_t.astype(BF),
            "sin2": sin_t.astype(BF),
            "p2": p2.astype(BF),
            "tri2x": tri2x.astype(BF),
        })
    return in_maps


def kernel(x, Wq, Wk, Wv, Wo, bo):
    global _BUILT, LAST_RESULT
    from concourse.bass_utils import run_bass_kernel_spmd

    if TRACE:
        _install_ntff_hook()

    if _BUILT is None:
        _BUILT = _build()
    nc = _BUILT

    in_maps = _host_inputs(x, Wq, Wk, Wv)
    Wo = np.asarray(Wo, dtype=np.float32)
    for c in range(N_CORES):
        in_maps[c]["wo"] = np.ascontiguousarray(
            Wo[128 * c : 128 * (c + 1), :]).astype(np.float16)

    last_err = None
    for attempt in range(3):
        try:
            res = run_bass_kernel_spmd(
                nc, in_maps, core_ids=list(range(N_CORES)), trace=TRACE
            )
            break
        except Exception as e:  # transient NRT device errors: retry
            last_err = e
            import time as _time

            _time.sleep(2.0)
    else:
        raise last_err
    LAST_RESULT = res

    acc = res.results[0]["y"].astype(np.float32)
    for c in range(1, N_CORES):
        acc = acc + res.results[c]["y"].astype(np.float32)
    out = acc + np.asarray(bo, dtype=np.float32)[None, :]
    return out.reshape(B, T, C)


# revision 33
# speedup vs baseline: 1.1119x; 1.0053x over previous
"""Causal multi-head self-attention (RoPE) Trainium2 Bass kernel.

Problem: x:(4,2048,1024), Wq/Wk/Wv:(1024,1024), Wo:(1024,1024), bo:(1024,)
  q,k,v = split_heads(x@W*), rope(q), rope(k), causal softmax(q k^T/8) v, @Wo+bo

Sharding: head-parallel across 8 cores. Core c owns heads {2c, 2c+1} for all
4 batches: it computes q/k/v projections against the 128-column weight slice,
attention for its heads, and a partial output projection against the matching
128-row slice of Wo. Host sums the 8 partial (8192,1024) fp16 outputs and
adds bo.

On-core layout:
  Q^T/K^T (128 x 2048/batch): rows = [h0 d-evens(32), h0 d-odds(32), h1 ...]
    (NeoX-style d-permutation, folded into the host-permuted weight columns;
     valid because q and k get the same permutation and qk^T is d-invariant)
  RoPE: Q <- P*cos - (P2@(P*sin2)), computed with DVE reads straight from the
    projection PSUM; P2 swaps even/odd halves per head (PE matmul), sin2
    carries the sign.
  V is projected token-major directly: per 128-token tile, lhsT = xT chunk,
    rhs = Wv chunk, accumulated over the 8 contraction chunks; PSUM result is
    cast once into Vb[tok, h, tile, 0:64], with a ones column at col 64
    (memset) for the softmax denominator.
  S^T tiles (tj x ti) = K^T.T @ Q^T per head (K=64 row-group pairs run
    concurrently on the PE).
  A = exp(0.125*S^T) (ACT, both heads in one instruction); straddle tiles get
    a post-exp fp16 0/1-triangle multiply (no PSUM read-modify-write).
  O~^T (65 x ti) accumulated = [V|1].T @ A over tj chunks; row 64 = softmax
    denominators. Normalize: PSUM->fp16 avs copy, SBUF-SBUF DMA repartition
    of the denominator row, lane-parallel reciprocal, repartition back to one
    row, GpSimd partition_broadcast, DVE multiply -> O^T (128 x 2048) fp16.
  y partial (128t x 1024) fp16 = O^T-chunk.T @ Wo-slice, evacuated half on
    DVE / half on ACT, DMA'd to DRAM as fp16.
"""

import numpy as np

B, T, C = 4, 2048, 1024
H, D = 16, 64
N_CORES = 8
BT = B * T
SCALE = 0.125  # D**-0.5

TRACE = False            # set True (e.g. from test.py) to capture an NTFF trace
LAST_RESULT = None       # BassKernelResults of the most recent run

_BUILT = None            # cached bass program


# --------------------------------------------------------------------------
# workaround: this walrus build rejects >1 semaphore wait per instruction
def _split_sem_waits(nc, max_waits=1):
    import concourse.mybir as mybir

    n = 0
    for f in nc.m.functions:
        for bb in f.blocks:
            insts = bb.instructions
            idx = 0
            while idx < len(insts):
                i = insts[idx]
                si = getattr(i, "sync_info", None)
                if si is not None and si.on_wait and len(si.on_wait) > max_waits:
                    waits = list(si.on_wait)
                    extra, keep = waits[:-max_waits], waits[-max_waits:]
                    si.on_wait = keep
                    pos = idx
                    for j in range(0, len(extra), max_waits):
                        n += 1
                        nd = mybir.InstNoOp(name=f"I-waitsplit-{n}", ins=[], outs=[])
                        nd.engine = i.engine
                        nd.sync_info = mybir.SyncInfo(
                            on_wait=extra[j : j + max_waits], on_update=[]
                        )
                        insts.insert(pos, nd)
                        pos += 1
                    idx = pos
                idx += 1


def _install_ntff_hook():
    """The image's antenv lacks axon_hooks; synthesize it so trace=True works."""
    import sys
    import types

    if "antenv.axon_hooks" in sys.modules:
        return
    import antenv

    state = {"hook": None}
    mod = types.ModuleType("antenv.axon_hooks")
    mod.get_axon_ntff_profile_hook = lambda: state["hook"]
    mod.set_axon_ntff_profile_hook = lambda h: state.__setitem__("hook", h)
    sys.modules["antenv.axon_hooks"] = mod
    antenv.axon_hooks = mod
    try:
        from trn_agent_boot.trn_boot import _ntff_profile_via_ctypes

        state["hook"] = _ntff_profile_via_ctypes("/opt/axon/libaxon_pjrt.so")
    except Exception:
        state["hook"] = None


# --------------------------------------------------------------------------
def _build():
    import concourse.bass as bass
    import concourse.mybir as mybir
    from concourse.tile import TileContext

    F = mybir.dt.float32
    MD = mybir.dt.float16  # matmul operand dtype
    MULT = mybir.AluOpType.mult
    SUB = mybir.AluOpType.subtract
    EXP = mybir.ActivationFunctionType.Exp

    nc = bass.Bass()

    xT = nc.dram_tensor("xT", (C, BT), MD, kind="ExternalInput")
    wq = nc.dram_tensor("wq", (C, 128), MD, kind="ExternalInput")
    wk = nc.dram_tensor("wk", (C, 128), MD, kind="ExternalInput")
    wv = nc.dram_tensor("wv", (C, 128), MD, kind="ExternalInput")
    wo = nc.dram_tensor("wo", (128, C), MD, kind="ExternalInput")
    cosd = nc.dram_tensor("cos", (128, T), MD, kind="ExternalInput")
    sind = nc.dram_tensor("sin2", (128, T), MD, kind="ExternalInput")
    p2d = nc.dram_tensor("p2", (128, 128), MD, kind="ExternalInput")
    trid = nc.dram_tensor("tri2x", (128, 256), MD, kind="ExternalInput")
    y = nc.dram_tensor("y", (BT, C), MD, kind="ExternalOutput")

    with TileContext(nc) as tc:
        with (
            tc.tile_pool(name="const", bufs=1) as cst,
            tc.tile_pool(name="xt", bufs=3) as xtp,
            tc.tile_pool(name="qt", bufs=2) as qp,
            tc.tile_pool(name="kt", bufs=2) as kp,
            tc.tile_pool(name="vt", bufs=2) as vp,
            tc.tile_pool(name="ot", bufs=2) as op_,
            tc.tile_pool(name="tmp", bufs=4) as tmp,
            tc.tile_pool(name="at", bufs=6) as ap_,
            tc.tile_pool(name="bc", bufs=4) as bcp,
            tc.tile_pool(name="avs", bufs=6) as avsp,
            tc.tile_pool(name="rr", bufs=12) as rp,
            tc.tile_pool(name="ys", bufs=4) as ysp,
            tc.tile_pool(name="sps", bufs=2, space="PSUM") as sps,
            tc.tile_pool(name="stp", bufs=2, space="PSUM") as stp,
            tc.tile_pool(name="avp", bufs=1, space="PSUM") as avp,
        ):
            # ---- constants (batched DMAs; ordered so the first projection
            # matmuls are not queued behind yet-unneeded loads) --------------
            wq_t = cst.tile([128, 8, 128], MD)
            wk_t = cst.tile([128, 8, 128], MD)
            wv_t = cst.tile([128, 8, 128], MD)
            nc.sync.dma_start(
                out=wq_t, in_=wq[:, :].rearrange("(k p) c -> p k c", p=128))
            # prefetch the first token block right behind wq so the first
            # projection matmuls are not queued behind other constant loads
            xt00 = xtp.tile([128, 8, 512], MD, name="xt")
            nc.sync.dma_start(
                out=xt00, in_=xT[:, 0:512].rearrange("(k p) c -> p k c", p=128))
            nc.sync.dma_start(
                out=wk_t, in_=wk[:, :].rearrange("(k p) c -> p k c", p=128))
            nc.sync.dma_start(
                out=wv_t, in_=wv[:, :].rearrange("(k p) c -> p k c", p=128))
            cos_t = cst.tile([128, T], MD)
            nc.sync.dma_start(out=cos_t, in_=cosd[:, :])
            sin_t = cst.tile([128, T], MD)
            nc.sync.dma_start(out=sin_t, in_=sind[:, :])
            p2_t = cst.tile([128, 128], MD)
            nc.sync.dma_start(out=p2_t, in_=p2d[:, :])
            tri_t = cst.tile([128, 256], MD)  # [tri01 | tri01] for head pairs
            nc.sync.dma_start(out=tri_t, in_=trid[:, :])
            ones64 = cst.tile([1, 64], MD)  # broadcast helper (lhsT)
            nc.vector.memset(ones64[0:1, :], 1.0)
            wo_t = cst.tile([128, C], MD)  # first needed ~i-block 1
            nc.sync.dma_start(out=wo_t, in_=wo[:, :])

            QKV = {}  # b -> (Qb, Kb, Vb)
            XT_PRE = {(0, 0): xt00}  # (b, nb) -> prefetched xt tile

            def xt_load(b, nb):
                g0 = b * T + nb * 512
                xt = xtp.tile([128, 8, 512], MD, name="xt")
                nc.sync.dma_start(
                    out=xt,
                    in_=xT[:, g0 : g0 + 512].rearrange("(k p) c -> p k c", p=128),
                )
                return xt

            def phase_a_alloc(b):
                Qb = qp.tile([128, T], MD, name="Qb")
                Kb = kp.tile([128, T], MD, name="Kb")
                # Vb token-major: per head 65 cols = [d 0..63 | ones]
                Vb = vp.tile([128, 2, 16, 65], MD, name="Vb")
                QKV[b] = (Qb, Kb, Vb)
                nc.vector.memset(Vb[:, :, :, 64], 1.0)

            def phase_a_unit(b, nb):
                Qb, Kb, Vb = QKV[b]
                cols = slice(nb * 512, (nb + 1) * 512)
                xt = XT_PRE.pop((b, nb), None)
                if xt is None:
                    xt = xt_load(b, nb)
                for W, dst in ((wq_t, Qb), (wk_t, Kb)):
                    ps = sps.tile([128, 512], F, tag="s", name="ps")
                    for k in range(8):
                        nc.tensor.matmul(
                            ps[:, :], lhsT=W[:, k, :], rhs=xt[:, k, :],
                            start=(k == 0), stop=(k == 7),
                        )
                    # rope: dst = ps*cos - P2@(ps*sin2)
                    #   (P2@ (q.sin2))[p] = -q~[p]*sin2[p], since sin2 is
                    #    antisymmetric and cos symmetric under the pair swap
                    qs = tmp.tile([128, 512], MD, name="qs")
                    nc.vector.tensor_tensor(qs[:, :], ps[:, :],
                                            sin_t[:, cols], MULT)
                    nc.vector.tensor_tensor(dst[:, cols], ps[:, :],
                                            cos_t[:, cols], MULT)
                    rot = sps.tile([128, 512], F, tag="s", name="rot")
                    nc.tensor.matmul(rot[:, :], lhsT=p2_t[:, :], rhs=qs[:, :],
                                     start=True, stop=True)
                    nc.vector.tensor_tensor(dst[:, cols], dst[:, cols],
                                            rot[:, :], SUB)
                # V token-major: per 128-token tile accumulate the 8 k-chunks
                for tl in range(4):
                    tt = nb * 4 + tl
                    vps = sps.tile([128, 128], F, tag="s", name="vps")
                    for k in range(8):
                        nc.tensor.matmul(
                            vps[:, :], lhsT=xt[:, k, tl * 128 : (tl + 1) * 128],
                            rhs=wv_t[:, k, :],
                            start=(k == 0), stop=(k == 7),
                        )
                    nc.vector.tensor_copy(
                        Vb[:, :, tt, 0:64],
                        vps[:, :].rearrange("p (h f) -> p h f", h=2),
                    )

            def y_unit(b, Ob, i):
                # output projection for the 4 token-tiles of ti-block i;
                # one batched 1 MB DMA for the whole 512-token block
                ysb = ysp.tile([128, 4, 1024], MD, name="ysb")
                for tl in range(4):
                    tt = 4 * i + tl
                    lhs = Ob[:, tt * 128 : (tt + 1) * 128]
                    for nh in (0, 1):
                        yps = sps.tile([128, 512], F, tag="s", name="yps")
                        nc.tensor.matmul(
                            yps[:, :], lhsT=lhs,
                            rhs=wo_t[:, nh * 512 : (nh + 1) * 512],
                            start=True, stop=True,
                        )
                        if nh == 0:
                            nc.vector.tensor_copy(
                                ysb[:, tl, 0:512], yps[:, :])
                        else:
                            nc.scalar.copy(ysb[:, tl, 512:1024], yps[:, :])
                r0 = b * T + i * 512
                nc.sync.dma_start(
                    out=y[r0 : r0 + 512, :].rearrange("(t p) c -> p t c", p=128),
                    in_=ysb[:, :, :],
                )

            def norm_evac(av):
                # evacuate both heads' accumulators (incl denom row 64) to
                # fp32 SBUF in one pass, releasing the PSUM banks promptly
                avs = avsp.tile([65, 2, 512], F, name="avs")
                nc.vector.tensor_copy(avs[:, :, :], av[0:65, :, :])
                return avs

            def norm_pre(avs):
                # denominators (both heads): SBUF-SBUF repartition ->
                # lane-parallel reciprocal -> fp16 -> repartition back to one
                # row. No PE instructions: emitted immediately so the DMA
                # latency runs while the deferred PE stage waits its turn.
                srt = rp.tile([128, 8], F, name="srt")
                nc.sync.dma_start(out=srt[:, :], in_=avs[64:65, :, :])
                rt = rp.tile([128, 8], F, name="rt")
                nc.vector.reciprocal(rt[:, :], srt[:, :])
                rth = rp.tile([128, 8], MD, name="rth")
                nc.vector.tensor_copy(rth[:, :], rt[:, :])
                rdr = rp.tile([1, 1024], MD, name="rdr")
                nc.sync.dma_start(out=rdr[0:1, :], in_=rth[:, :])
                return rdr

            def norm_fin(rdr, avs, Ob, i, h):
                # PE ones-matmul broadcast into 64 PSUM partitions, then
                # normalize into Ob
                bct = sps.tile([128, 512], F, tag="s", name="bct")
                nc.tensor.matmul(
                    bct[0:64, :], lhsT=ones64[0:1, :],
                    rhs=rdr[0:1, h * 512 : (h + 1) * 512],
                    start=True, stop=True,
                )
                nc.vector.tensor_tensor(
                    Ob[64 * h : 64 * h + 64, i * 512 : (i + 1) * 512],
                    avs[0:64, h, :], bct[0:64, :], MULT,
                )

            def phase_d(b, filler=None, pre=None, carry=()):
                Qb, Kb, Vb = QKV[b]
                Ob = op_.tile([128, T], MD, name="Ob")
                defer = list(carry)  # deferred PE-stage closures (norm_fin /
                # y_unit), emitted one per j-slot a ti-block later so the PE
                # queue never heads on a normalize DMA
                for i in range(4):
                    if pre is not None:
                        pre(i)
                    av = avp.tile([128, 2, 512], F, tag="av", name="av")
                    nch = 4 * i + 4
                    sts = {}

                    def emit_st(j):
                        delta = j * 128 - i * 512
                        nl = 512 - max(0, delta)
                        off = 512 - nl
                        st = stp.tile([128, 2, 512], F, name="st")
                        for h in (0, 1):
                            hs = slice(64 * h, 64 * h + 64)
                            nc.tensor.matmul(
                                st[:, h, 0:nl],
                                lhsT=Kb[hs, j * 128 : (j + 1) * 128],
                                rhs=Qb[hs, i * 512 + off : (i + 1) * 512],
                                start=True, stop=True,
                            )
                        sts[j] = (st, off, nl, delta >= 0)

                    LAG = 1
                    for j in range(min(LAG, nch)):
                        emit_st(j)
                    for j in range(nch):
                        if j + LAG < nch:
                            emit_st(j + LAG)
                        if defer and j in (1, 2, 3):
                            defer.pop(0)()
                        st, off, nl, straddle = sts.pop(j)
                        A = ap_.tile([128, 2, 512], MD, name="A")
                        nc.scalar.activation(
                            A[:, :, 0:nl], st[:, :, 0:nl], EXP, scale=SCALE)
                        if straddle:  # mask the diagonal triangle (post-exp)
                            # on GpSimd: it is otherwise idle and this keeps
                            # the hot DVE out of the A critical path
                            nc.gpsimd.tensor_tensor(
                                A[:, :, 0:128], A[:, :, 0:128],
                                tri_t[:, :].rearrange("p (a c) -> p a c", a=2),
                                MULT)
                        for h in (0, 1):
                            nc.tensor.matmul(
                                av[0:65, h, off:512],
                                lhsT=Vb[:, h, j, :],
                                rhs=A[:, h, 0:nl],
                                start=(j == 0), stop=(j == nch - 1),
                                skip_group_check=True,
                            )
                    avs = norm_evac(av)
                    rdr = norm_pre(avs)
                    if b > 1 and i < 3:
                        # steady state: inline — the chain hides under
                        # subsequent blocks and keeps PE duty moderate
                        for h in (0, 1):
                            norm_fin(rdr, avs, Ob, i, h)
                        defer.append(lambda i2=i: y_unit(b, Ob, i2))
                    else:
                        # batch 0 (PE runs caught-up with emission, any
                        # serial chain is a PE stall) and each batch's last
                        # i-block: defer the PE stage + y_unit into the
                        # subsequent emission stream
                        defer.extend([
                            lambda r=rdr, a=avs, i2=i, h2=h:
                                norm_fin(r, a, Ob, i2, h2)
                            for h in (0, 1)
                        ])
                        defer.append(lambda i2=i: y_unit(b, Ob, i2))
                    if filler is not None:
                        filler(i)
                if b == B - 1:
                    for fn in defer:
                        fn()
                    defer = []
                return defer

            phase_a_alloc(0)
            carry = ()
            for b in range(B):
                if b + 1 < B:
                    phase_a_alloc(b + 1)
                    fil = (lambda i, nb=b + 1: phase_a_unit(nb, i))
                else:
                    fil = None
                # batch 0's projection blocks are emitted just-in-time ahead
                # of the attention block that first needs them
                pre = (lambda i: phase_a_unit(0, i)) if b == 0 else None
                carry = phase_d(b, filler=fil, pre=pre, carry=carry)

    _split_sem_waits(nc)
    return nc


# --------------------------------------------------------------------------
def _host_inputs(x, Wq, Wk, Wv):
    """Per-core input dicts (all shared arrays built once)."""
    BF = np.float16
    xT = np.ascontiguousarray(
        np.asarray(x, dtype=np.float32).reshape(BT, C).T).astype(BF)

    # NeoX d-permutation within each head: evens then odds
    dperm = np.concatenate([np.arange(0, D, 2), np.arange(1, D, 2)])

    inv_freq = (1.0 / (10000.0 ** (np.arange(0, D, 2) / D))).astype(np.float64)
    pos = np.arange(T, dtype=np.float64)
    ang = pos[None, :] * inv_freq[:, None]  # (32, T)
    cos32 = np.cos(ang).astype(np.float32)
    sin32 = np.sin(ang).astype(np.float32)
    cos_t = np.tile(np.vstack([cos32, cos32]), (2, 1))  # (128, T)
    sin_t = np.tile(np.vstack([-sin32, sin32]), (2, 1))  # (128, T), sign folded

    p2 = np.zeros((128, 128), dtype=np.float32)
    for hb in (0, 64):
        for i2 in range(32):
            p2[hb + i2, hb + 32 + i2] = 1.0
            p2[hb + 32 + i2, hb + i2] = 1.0

    # 0/1 mask: 0 where k-token (partition) > q-token (col), else 1
    tri = np.where(
        np.arange(128)[None, :] < np.arange(128)[:, None], 0.0, 1.0
    ).astype(np.float32)
    tri2x = np.concatenate([tri, tri], axis=1)  # (128, 256)

    Wq = np.asarray(Wq, dtype=np.float32)
    Wk = np.asarray(Wk, dtype=np.float32)
    Wv = np.asarray(Wv, dtype=np.float32)

    in_maps = []
    for c in range(N_CORES):
        sl = slice(128 * c, 128 * (c + 1))
        wq_c = Wq[:, sl].reshape(C, 2, D)[:, :, dperm].reshape(C, 128)
        wk_c = Wk[:, sl].reshape(C, 2, D)[:, :, dperm].reshape(C, 128)
        in_maps.append({
            "xT": xT,
            "wq": np.ascontiguousarray(wq_c).astype(BF),
            "wk": np.ascontiguousarray(wk_c).astype(BF),
            "wv": np.ascontiguousarray(Wv[:, sl]).astype(BF),
            "wo": None,  # set below
            "cos": cos_t.astype(BF),
            "sin2": sin_t.astype(BF),
            "p2": p2.astype(BF),
            "tri2x": tri2x.astype(BF),
        })
    return in_maps


def kernel(x, Wq, Wk, Wv, Wo, bo):
    global _BUILT, LAST_RESULT
    from concourse.bass_utils import run_bass_kernel_spmd

    if TRACE:
        _install_ntff_hook()

    if _BUILT is None:
        _BUILT = _build()
    nc = _BUILT

    in_maps = _host_inputs(x, Wq, Wk, Wv)
    Wo = np.asarray(Wo, dtype=np.float32)
    for c in range(N_CORES):
        in_maps[c]["wo"] = np.ascontiguousarray(
            Wo[128 * c : 128 * (c + 1), :]).astype(np.float16)

    last_err = None
    for attempt in range(3):
        try:
            res = run_bass_kernel_spmd(
                nc, in_maps, core_ids=list(range(N_CORES)), trace=TRACE
            )
            break
        except Exception as e:  # transient NRT device errors: retry
            last_err = e
            import time as _time

            _time.sleep(2.0)
    else:
        raise last_err
    LAST_RESULT = res

    acc = res.results[0]["y"].astype(np.float32)
    for c in range(1, N_CORES):
        acc = acc + res.results[c]["y"].astype(np.float32)
    out = acc + np.asarray(bo, dtype=np.float32)[None, :]
    return out.reshape(B, T, C)
